# revision 1
# baseline (speedup 1.0000x reference)
"""TRN2 Bass kernel for nn_LoRA_80839874445852.

Computes out = x @ W^T + b + (x @ A_b) @ B_b / 16 for bs=8, sq=2048, d=4096,
r=16 (per-batch LoRA skill blocks), distributed data-parallel over the batch
dim across 8 NeuronCores.

Strategy:
  * (x @ A) @ B == x @ (A @ B): the rank-16 LoRA term is folded into the
    weight on the host (cheap: 4 distinct [4096,16]@[16,4096] products), so
    each core runs a single dense GEMM:  outT = W_eff^T-contraction with xT.
  * Per core i:  W_eff_i = W.T + A_flat[i//2] @ B_flat[i//2] / 16  [d, o]
  * Device computes outT[o, s] = sum_d W_eff[d, o] * xT[d, s] tiled as
    128x128 stationary (W_eff) x 128x512 moving (xT) bf16 matmuls with fp32
    PSUM accumulation over d (32 tiles), + per-partition bias add on evict.
  * x is resident in SBUF (bf16, 16MB); W_eff streamed once (32x1MB blocks).
  * Host transposes outT back and stacks the full [8, 2048, 4096] output.

bf16 inputs + fp32 accumulate give ~1.6e-3 relative error vs the fp32
reference (error dominated by the input cast; engine accumulation is ~1e-7).
"""
import numpy as np
import ml_dtypes

import concourse.bacc as bacc
import concourse.tile as tile
import concourse.mybir as mybir
from concourse.bass_utils import run_bass_kernel_spmd

# Problem shape (hardcoded per spec)
BS, SQ, D = 8, 2048, 4096
R = 16
N_CORES = 8
P = 128
ND = D // P      # 32 d-tiles (contraction)
NO = D // P      # 32 o-blocks (output features)
NS = 512         # moving free dim per matmul (PSUM bank = 512 fp32)
NST = SQ // NS   # 4 s-tiles

BF16 = mybir.dt.bfloat16
F32 = mybir.dt.float32

_CACHED = {}


def _build():
    """Build + compile the per-core Bass program (same program, all cores)."""
    nc = bacc.Bacc("TRN2", target_bir_lowering=False, debug=False)
    x_d = nc.dram_tensor("x", [ND, P, SQ], BF16, kind="ExternalInput").ap()
    w_d = nc.dram_tensor("w", [NO, P, ND, P], BF16, kind="ExternalInput").ap()
    b_d = nc.dram_tensor("b", [P, NO], F32, kind="ExternalInput").ap()
    y_d = nc.dram_tensor("y", [NO, P, SQ], F32, kind="ExternalOutput").ap()

    with tile.TileContext(nc) as tc:
        with (
            tc.tile_pool(name="xpool", bufs=1) as xpool,
            tc.tile_pool(name="wpool", bufs=3) as wpool,
            tc.tile_pool(name="opool", bufs=3) as opool,
            tc.tile_pool(name="cpool", bufs=1) as cpool,
            tc.tile_pool(name="psum", bufs=2, space="PSUM") as psum_pool,
        ):
            bias_t = cpool.tile([P, NO], F32)
            nc.sync.dma_start(out=bias_t[:], in_=b_d[:])

            # resident x: 32 tiles [128, 2048] bf16 (512KB each)
            xts = []
            for dtile in range(ND):
                xt = xpool.tile([P, SQ], BF16, tag=f"x{dtile}")
                nc.sync.dma_start(out=xt[:], in_=x_d[dtile])
                xts.append(xt)

            for o in range(NO):
                wt = wpool.tile([P, ND, P], BF16, tag="w")
                nc.sync.dma_start(out=wt[:], in_=w_d[o])
                ps = psum_pool.tile([P, NST, NS], F32, tag="ps")
                for d in range(ND):
                    for s_t in range(NST):
                        nc.tensor.matmul(
                            ps[:, s_t],
                            lhsT=wt[:, d],
                            rhs=xts[d][:, s_t * NS:(s_t + 1) * NS],
                            start=(d == 0),
                            stop=(d == ND - 1),
                        )
                ot = opool.tile([P, NST, NS], F32, tag="o")
                nc.vector.tensor_scalar_add(ot[:], ps[:], bias_t[:, o:o + 1])
                nc.sync.dma_start(
                    out=y_d[o], in_=ot[:].rearrange("p nst ns -> p (nst ns)")
                )
    nc.compile()
    return nc


def _prep_inputs(x, W, b, A, B):
    """Host-side shard + layout prep. Returns per-core input maps."""
    x = np.asarray(x, dtype=np.float32)
    W = np.asarray(W, dtype=np.float32)
    b = np.asarray(b, dtype=np.float32)
    A = np.asarray(A, dtype=np.float32)
    B = np.asarray(B, dtype=np.float32)

    n_splits = A.shape[0]
    repeat = BS // n_splits  # 2

    # bias: [128, 32] with bh[p, o] = b[o*128 + p]
    bh = np.ascontiguousarray(b.reshape(NO, P).T)

    # distinct folded weights per skill group, in SBUF tile layout
    w_maps = []
    for g in range(n_splits):
        A_flat = A[g].reshape(D, R)                     # [d, r]
        B_flat = B[g].transpose(1, 0, 2).reshape(R, D)  # [r, o]
        W_eff = W.T + (A_flat @ B_flat) * (1.0 / R)     # [d, o]
        # Wh[o_t, p, do, q] = W_eff[do*128+p, o_t*128+q]
        Wh = np.ascontiguousarray(
            W_eff.reshape(ND, P, NO, P).transpose(2, 1, 0, 3)
        ).astype(ml_dtypes.bfloat16)
        w_maps.append(Wh)

    in_maps = []
    for i in range(BS):
        # xh[do, p, s] = x[i][s, do*128+p]
        xh = np.ascontiguousarray(x[i].T.reshape(ND, P, SQ)).astype(
            ml_dtypes.bfloat16
        )
        in_maps.append({"x": xh, "w": w_maps[i // repeat], "b": bh})
    return in_maps


def kernel(x, W, b, A, B):
    if "nc" not in _CACHED:
        _CACHED["nc"] = _build()
    nc = _CACHED["nc"]

    in_maps = _prep_inputs(x, W, b, A, B)
    res = run_bass_kernel_spmd(nc, in_maps, list(range(N_CORES)))

    out = np.empty((BS, SQ, D), dtype=np.float32)
    for i in range(BS):
        yT = res.results[i]["y"].reshape(D, SQ)  # outT [o, s]
        out[i] = yT.T
    return out



# revision 2
# speedup vs baseline: 2.0034x; 2.0034x over previous
"""TRN2 Bass kernel for nn_LoRA_80839874445852.

Computes out = x @ W^T + b + (x @ A_b) @ B_b / 16 for bs=8, sq=2048, d=4096,
r=16 (per-batch LoRA skill blocks), distributed data-parallel over the batch
dim across 8 NeuronCores.

Strategy:
  * (x @ A) @ B == x @ (A @ B): the rank-16 LoRA term is folded into the
    weight on the host (cheap: 4 distinct [4096,16]@[16,4096] products), so
    each core runs a single dense GEMM:  outT = W_eff^T-contraction with xT.
  * Per core i:  W_eff_i = W.T + A_flat[i//2] @ B_flat[i//2] / 16  [d, o]
  * Device computes outT[o, s] = sum_d W_eff[d, o] * xT[d, s] tiled as
    128x128 stationary (W_eff) x 128x512 moving (xT) bf16 matmuls with fp32
    PSUM accumulation over d (32 tiles), + per-partition bias add on evict.
  * x is resident in SBUF (bf16, 16MB); W_eff streamed once (32x1MB blocks).
  * PSUM eviction is per-bank (EVICT_BANKS=1): each 512-column accumulation
    group drains to SBUF + HBM as soon as its last matmul retires, keeping
    up to 8 PSUM banks in flight so the PE never waits on the evict path.
  * Host transposes outT back and stacks the full [8, 2048, 4096] output.

bf16 inputs + fp32 accumulate give ~2e-3 relative error vs the fp32
reference (error dominated by the input cast; engine accumulation is ~1e-7).
"""
import numpy as np
import ml_dtypes

import concourse.bacc as bacc
import concourse.tile as tile
import concourse.mybir as mybir
from concourse.bass_utils import run_bass_kernel_spmd

# Problem shape (hardcoded per spec)
BS, SQ, D = 8, 2048, 4096
R = 16
N_CORES = 8
P = 128
ND = D // P      # 32 d-tiles (contraction)
NO = D // P      # 32 o-blocks (output features)
NS = 512         # moving free dim per matmul (PSUM bank = 512 fp32)
NST = SQ // NS   # 4 s-tiles

EVICT_BANKS = 1  # PSUM banks per eviction group (1, 2, or 4)

BF16 = mybir.dt.bfloat16
F32 = mybir.dt.float32

_CACHED = {}


def _build(reps=1):
    """Build + compile the per-core Bass program (same program, all cores).

    reps > 1 repeats the steady-state o-loop for repetition-slope timing;
    the computed output is identical (each rep overwrites y).
    """
    eb = EVICT_BANKS
    n_groups = NST // eb
    nc = bacc.Bacc("TRN2", target_bir_lowering=False, debug=False)
    x_d = nc.dram_tensor("x", [ND, P, SQ], BF16, kind="ExternalInput").ap()
    w_d = nc.dram_tensor("w", [NO, P, ND, P], BF16, kind="ExternalInput").ap()
    b_d = nc.dram_tensor("b", [P, NO], F32, kind="ExternalInput").ap()
    y_d = nc.dram_tensor("y", [NO, P, SQ], F32, kind="ExternalOutput").ap()

    with tile.TileContext(nc) as tc:
        with (
            tc.tile_pool(name="xpool", bufs=1) as xpool,
            tc.tile_pool(name="wpool", bufs=3) as wpool,
            tc.tile_pool(name="opool", bufs=2 * n_groups) as opool,
            tc.tile_pool(name="cpool", bufs=1) as cpool,
            tc.tile_pool(name="psum", bufs=8 // eb, space="PSUM") as psum_pool,
        ):
            bias_t = cpool.tile([P, NO], F32)
            nc.sync.dma_start(out=bias_t[:], in_=b_d[:])

            # resident x: 32 tiles [128, 2048] bf16 (512KB each)
            xts = []
            for dtile in range(ND):
                xt = xpool.tile([P, SQ], BF16, tag=f"x{dtile}")
                nc.sync.dma_start(out=xt[:], in_=x_d[dtile])
                xts.append(xt)

            for rep in range(reps):
                for o in range(NO):
                    wt = wpool.tile([P, ND, P], BF16, tag="w")
                    nc.sync.dma_start(out=wt[:], in_=w_d[o])
                    for g in range(n_groups):
                        ps = psum_pool.tile([P, eb, NS], F32, tag="ps")
                        for d in range(ND):
                            for j in range(eb):
                                s_t = g * eb + j
                                nc.tensor.matmul(
                                    ps[:, j],
                                    lhsT=wt[:, d],
                                    rhs=xts[d][:, s_t * NS:(s_t + 1) * NS],
                                    start=(d == 0),
                                    stop=(d == ND - 1),
                                )
                        ot = opool.tile([P, eb, NS], F32, tag="o")
                        nc.vector.tensor_scalar_add(
                            ot[:], ps[:], bias_t[:, o:o + 1]
                        )
                        nc.sync.dma_start(
                            out=y_d[o][:, g * eb * NS:(g + 1) * eb * NS],
                            in_=ot[:].rearrange("p a b -> p (a b)"),
                        )
    nc.compile()
    return nc


def _prep_inputs(x, W, b, A, B):
    """Host-side shard + layout prep. Returns per-core input maps."""
    x = np.asarray(x, dtype=np.float32)
    W = np.asarray(W, dtype=np.float32)
    b = np.asarray(b, dtype=np.float32)
    A = np.asarray(A, dtype=np.float32)
    B = np.asarray(B, dtype=np.float32)

    n_splits = A.shape[0]
    repeat = BS // n_splits  # 2

    # bias: [128, 32] with bh[p, o] = b[o*128 + p]
    bh = np.ascontiguousarray(b.reshape(NO, P).T)

    # distinct folded weights per skill group, in SBUF tile layout
    w_maps = []
    for g in range(n_splits):
        A_flat = A[g].reshape(D, R)                     # [d, r]
        B_flat = B[g].transpose(1, 0, 2).reshape(R, D)  # [r, o]
        W_eff = W.T + (A_flat @ B_flat) * (1.0 / R)     # [d, o]
        # Wh[o_t, p, do, q] = W_eff[do*128+p, o_t*128+q]
        Wh = np.ascontiguousarray(
            W_eff.reshape(ND, P, NO, P).transpose(2, 1, 0, 3)
        ).astype(ml_dtypes.bfloat16)
        w_maps.append(Wh)

    in_maps = []
    for i in range(BS):
        # xh[do, p, s] = x[i][s, do*128+p]
        xh = np.ascontiguousarray(x[i].T.reshape(ND, P, SQ)).astype(
            ml_dtypes.bfloat16
        )
        in_maps.append({"x": xh, "w": w_maps[i // repeat], "b": bh})
    return in_maps


def kernel(x, W, b, A, B):
    if "nc" not in _CACHED:
        _CACHED["nc"] = _build()
    nc = _CACHED["nc"]

    in_maps = _prep_inputs(x, W, b, A, B)
    res = run_bass_kernel_spmd(nc, in_maps, list(range(N_CORES)))

    out = np.empty((BS, SQ, D), dtype=np.float32)
    for i in range(BS):
        yT = res.results[i]["y"].reshape(D, SQ)  # outT [o, s]
        out[i] = yT.T
    return out
